# revision 4
# baseline (speedup 1.0000x reference)
"""LocalMHA (windowed attention, window=128, look_backward=1, RoPE) on 8 TRN2 cores.

Sharding: sequence-parallel, no collectives. Core c handles batch c//2,
sequence half c%2 (2048 query tokens + a 128-token look-backward halo whose
x rows ride along in the core's input shard; zeros at a true sequence start,
where the mask kills the backward keys anyway).

v2 redesign vs the DRAM-staged baseline:
- Everything stages in SBUF (qT/kT/v as bf16) - no qrope/k2/vstage DRAM
  round-trips.
- Attention is computed in transposed-score form: scores^T [keys, queries]
  come straight out of the PE with kT as the stationary operand (one
  64-contraction matmul per head x key-slice, head A at partition base 0,
  head B at base 64 - the only bases the hardware path supports). Softmax:
  exp on ACT (batched per 2-head score bank), the {0,1} band mask is applied
  multiplicatively during the bf16 eviction (one DVE op - no mask-add, no
  -1e9 constants), and the denominators come for free from a ones-column
  appended to v in the AV matmul. The AV output lands in natural
  [query, dim] orientation so the normalizer is a per-partition
  tensor_scalar multiply. No probability transposes.
- RoPE: k is roped once at angle (t mod 128); q is roped once at angle i
  (pairs with current-window keys) and the look-backward phase
  q_hi = R(128) q_lo is derived per window with per-partition-scalar
  multiplies (rotations about the same planes commute, so the fixed extra
  rotation commutes with the position-dependent one). All rope arithmetic
  is bf16 at 2x DVE rate; PSUM->SBUF bf16 evictions run on ACT.
- Layout is head-contiguous (natural W_qkv column order), so the rotate_half
  partner is r^32 within each head's 64 rows: the sin multiply runs as four
  quarter-width partition-shifted ops, sin sign folded host-side.
"""

import numpy as np
from contextlib import ExitStack

import concourse.bacc as bacc
import concourse.tile as tile
import concourse.mybir as mybir
from concourse.bass_utils import run_bass_kernel_spmd
from concourse.masks import make_identity

# Problem shape (hardcoded per contract)
B, N, D = 4, 4096, 1024
H, DH, WS = 16, 64, 128
THETA = 10000.0
N3 = 3 * H * DH            # 3072
NCORES = 8
HALF = N // 2              # 2048 query tokens per core
NT = HALF + WS             # 2176 tokens incl halo window
NWIN = HALF // WS          # 16 query windows
NBLK = NT // WS            # 17 key blocks
SCALE = DH ** -0.5

F32 = mybir.dt.float32
F32R = mybir.dt.float32r
BF16 = mybir.dt.bfloat16
ADD = mybir.AluOpType.add
MUL = mybir.AluOpType.mult
EXP = mybir.ActivationFunctionType.Exp

# token chunks (start, len); 128-aligned, len<=512
CHUNKS = [(0, 512), (512, 512), (1024, 512), (1536, 512), (2048, 128)]

# rotate_half partner fragments (dst_base, src_base), partner = r^32 per head
_FRAGS = [(0, 32), (32, 0), (64, 96), (96, 64)]


def _rope(nc, tmpp, src, dst, L, rp, ci, si):
    """dst[:, :L] = src*cos + rot32(src)*sin_signed, all bf16 (2x DVE rate).

    Head-contiguous layout: rotate partner of row r is r^32 within each
    64-row head. The sin tile is indexed by SOURCE row with the
    destination's sign folded in host-side."""
    t1 = tmpp.tile([128, 512], BF16, tag="t1")
    nc.vector.tensor_tensor(t1[:, :L], src[:, :L], rp[:, ci, :L], MUL)
    t2 = tmpp.tile([128, 512], BF16, tag="t2")
    for (d0, s0) in _FRAGS:
        nc.vector.tensor_tensor(t2[d0:d0 + 32, :L], src[s0:s0 + 32, :L],
                                rp[s0:s0 + 32, si, :L], MUL)
    nc.vector.tensor_tensor(dst[:, :L], t1[:, :L], t2[:, :L], ADD)


def _build(reps=1):
    nc = bacc.Bacc("TRN2", target_bir_lowering=False, debug=False,
                   enable_asserts=False, num_devices=NCORES)

    xs = nc.dram_tensor("xs", [NT, D], F32R, kind="ExternalInput").ap()
    wqkv = nc.dram_tensor("wqkv", [D, N3], BF16, kind="ExternalInput").ap()
    wout = nc.dram_tensor("wout", [D, D], BF16, kind="ExternalInput").ap()
    # 0:qcos 1:qsin 2:kcos 3:ksin  (q tiles carry the 1/sqrt(dh) scale)
    ropes = nc.dram_tensor("ropes", [4, 128, 512], BF16, kind="ExternalInput").ap()
    # fixed R(128) rotation scalars: 0:cos 1:sin (source-signed)
    r128 = nc.dram_tensor("r128", [2, 128, 1], F32, kind="ExternalInput").ap()
    # {0,1} band masks: [variant, j, head2, slice, i]
    masks = nc.dram_tensor("masks", [2, 128, 2, 2, 128], BF16,
                           kind="ExternalInput").ap()
    out = nc.dram_tensor("out", [HALF, D], F32, kind="ExternalOutput").ap()

    with tile.TileContext(nc) as tc:
        with ExitStack() as top:
            constp = top.enter_context(tc.tile_pool(name="const", bufs=1))
            identf = constp.tile([128, 128], F32, tag="idf")
            make_identity(nc, identf[:])
            identb = constp.tile([128, 128], BF16, tag="idb")
            nc.vector.tensor_copy(identb[:], identf[:])
            identr = constp.tile([128, 128], F32R, tag="idr")
            nc.vector.tensor_copy(identr[:], identf[:])
            rp = constp.tile([128, 4, 512], BF16, tag="ropes")
            nc.sync.dma_start(rp[:], ropes.rearrange("r p m -> p r m"))
            rc = constp.tile([128, 2, 1], F32, tag="r128")
            nc.sync.dma_start(rc[:], r128.rearrange("r p o -> p r o"))
            mk = constp.tile([128, 2, 2, 2, 128], BF16, tag="masks")
            nc.sync.dma_start(mk[:], masks.rearrange("v p h s i -> p v h s i"))

            rep_ctx = tc.For_i(0, reps, 1) if reps > 1 else ExitStack()
            top.enter_context(rep_ctx)

            with ExitStack() as stg:
                stgp = stg.enter_context(tc.tile_pool(name="stg", bufs=1))
                qlo = stgp.tile([128, 8, HALF], BF16, tag="qlo")
                kT = stgp.tile([128, 8, 8 * WS], BF16, tag="kT")
                vp = stgp.tile([128, 8, 16, 65], BF16, tag="vp")
                nc.vector.memset(vp[:, :, :, 64:65], 1.0)

                # ---- fused pipeline: chunks interleaved with windows ----
                wp = stg.enter_context(tc.tile_pool(name="wq", bufs=1))
                w_sb = wp.tile([128, 8, N3], BF16, tag="w")
                nc.sync.dma_start(w_sb[:],
                                  wqkv.rearrange("(c p) n -> p c n", p=128))
                wop = stg.enter_context(tc.tile_pool(name="wo", bufs=1))
                wo = wop.tile([128, 8, D], BF16, tag="wo")
                nc.sync.dma_start(wo[:],
                                  wout.rearrange("(c p) n -> p c n", p=128))
                xp = stg.enter_context(tc.tile_pool(name="xst", bufs=4))
                xtp = stg.enter_context(tc.tile_pool(name="xT", bufs=2))
                rawp = stg.enter_context(tc.tile_pool(name="raw", bufs=2))
                tmpp = stg.enter_context(tc.tile_pool(name="tmp", bufs=2))
                qhp = stg.enter_context(tc.tile_pool(name="qh", bufs=2))
                qtp = stg.enter_context(tc.tile_pool(name="qt", bufs=2))
                eep = stg.enter_context(tc.tile_pool(name="ee", bufs=2))
                ptp = stg.enter_context(tc.tile_pool(name="pt", bufs=2))
                rrp = stg.enter_context(tc.tile_pool(name="rr", bufs=4))
                asp = stg.enter_context(tc.tile_pool(name="as", bufs=1))
                atp = stg.enter_context(tc.tile_pool(name="at", bufs=2))
                outp = stg.enter_context(tc.tile_pool(name="osb", bufs=2))
                # PSUM budget (8 banks): mm 3, s 2, av 1, aq 1, op 1
                mps = stg.enter_context(tc.tile_pool(name="mps", bufs=3,
                                                     space="PSUM"))
                sps = stg.enter_context(tc.tile_pool(name="sps", bufs=2,
                                                     space="PSUM"))
                avp = stg.enter_context(tc.tile_pool(name="avp", bufs=1,
                                                     space="PSUM"))
                obp = stg.enter_context(tc.tile_pool(name="obp", bufs=1,
                                                     space="PSUM"))

                def emit_chunk(s, L):
                    nmt = L // 128
                    slot0 = (s // 128) % 8
                    xT = xtp.tile([128, 8, 512], BF16, tag="xT")
                    xsts = []
                    for mt in range(nmt):
                        xst = xp.tile([128, D], F32R, tag="x")
                        nc.sync.dma_start(
                            xst[:], xs[s + mt * 128: s + (mt + 1) * 128, :])
                        xsts.append(xst)
                    for kc in range(8):
                        tp = mps.tile([128, 512], F32R, tag="mm")
                        for mt in range(nmt):
                            nc.tensor.transpose(
                                tp[:, mt * 128:(mt + 1) * 128],
                                xsts[mt][:, kc * 128:(kc + 1) * 128],
                                identr[:])
                        nc.scalar.copy(xT[:, kc, 0:L], tp[:, 0:L])

                    # q^T (+rope at angle i) for query tokens of this chunk
                    qs = 128 if s == 0 else 0
                    qL = L - qs
                    if qL > 0:
                        for nch in range(8):
                            qp = mps.tile([128, 512], F32, tag="mm")
                            for kc in range(8):
                                nc.tensor.matmul(
                                    qp[:, :qL],
                                    w_sb[:, kc, nch * 128:(nch + 1) * 128],
                                    xT[:, kc, qs:qs + qL],
                                    start=(kc == 0), stop=(kc == 7))
                            raw = rawp.tile([128, 512], BF16, tag="raw")
                            nc.scalar.copy(raw[:, :qL], qp[:, :qL])
                            q0 = s + qs - 128
                            _rope(nc, tmpp, raw,
                                  qlo[:, nch, q0:q0 + qL], qL, rp, 0, 1)

                    # k^T (roped once at angle t%128) into the 8-block ring
                    for nch in range(8):
                        kp = mps.tile([128, 512], F32, tag="mm")
                        for kc in range(8):
                            nc.tensor.matmul(
                                kp[:, :L],
                                w_sb[:, kc, 1024 + nch * 128:
                                     1024 + (nch + 1) * 128],
                                xT[:, kc, 0:L],
                                start=(kc == 0), stop=(kc == 7))
                        raw = rawp.tile([128, 512], BF16, tag="raw")
                        nc.scalar.copy(raw[:, :L], kp[:, :L])
                        _rope(nc, tmpp, raw,
                              kT[:, nch, slot0 * 128: slot0 * 128 + L],
                              L, rp, 2, 3)

                    # v in natural layout (+ones col), bf16, 8-block ring
                    for mt in range(nmt):
                        wt = (s // 128 + mt) % 8
                        for nh in range(2):
                            vq = mps.tile([128, 512], F32, tag="mm")
                            for kc in range(8):
                                nc.tensor.matmul(
                                    vq[:],
                                    xT[:, kc, mt * 128:(mt + 1) * 128],
                                    w_sb[:, kc, 2048 + nh * 512:
                                         2048 + (nh + 1) * 512],
                                    start=(kc == 0), stop=(kc == 7))
                            nc.scalar.copy(
                                vp[:, wt, nh * 8:(nh + 1) * 8, 0:64],
                                vq[:].rearrange("p (h e) -> p h e", e=64))

                def emit_window(b):
                    q0 = (b - 1) * 128
                    # q_hi = R(128) q_lo : per-partition scalar rotation
                    qhi = qhp.tile([128, 8, 128], BF16, tag="qhi")
                    t1 = qtp.tile([128, 8, 128], BF16, tag="qt1")
                    nc.vector.tensor_scalar_mul(
                        t1[:], qlo[:, :, q0:q0 + 128], rc[:, 0, :])
                    t2 = qtp.tile([128, 8, 128], BF16, tag="qt2")
                    for (d0, s0) in _FRAGS:
                        nc.vector.tensor_scalar_mul(
                            t2[d0:d0 + 32], qlo[s0:s0 + 32, :, q0:q0 + 128],
                            rc[s0:s0 + 32, 1, :])
                    nc.vector.tensor_tensor(qhi[:], t1[:], t2[:], ADD)

                    var = 0 if b == 1 else 1
                    a_sb = asp.tile([128, 1024], BF16, tag="a")
                    for g in range(8):
                        # scores^T: one 64-contraction matmul per
                        # (head, key-slice); one PSUM bank per head (matmuls
                        # at different partition bases run on different PE
                        # row groups concurrently - same-bank drains crash).
                        ee = eep.tile([128, 2, 2, 128], BF16, tag="ee")
                        sptiles = []
                        for h2 in range(2):
                            r0 = h2 * 64
                            sp = sps.tile([128, 2, 128], F32, tag="s")
                            sptiles.append(sp)
                            for s_ in range(2):
                                rhq = qhi[:, g, :] if s_ == 0 \
                                    else qlo[:, g, q0:q0 + 128]
                                kslot = (b - 1 + s_) % 8
                                nc.tensor.matmul(
                                    sp[:, s_, :],
                                    kT[r0:r0 + 64, g,
                                       kslot * 128:(kslot + 1) * 128],
                                    rhq[r0:r0 + 64, :],
                                    start=True, stop=True)
                        for h2 in range(2):
                            nc.scalar.activation(ee[:, h2], sptiles[h2][:],
                                                 EXP)
                        pT = ptp.tile([128, 2, 2, 128], BF16, tag="pT")
                        nc.vector.tensor_tensor(pT[:], ee[:],
                                                mk[:, var], MUL)
                        # AV in natural [query, dim] orientation; col 64 of v
                        # is ones -> col 64 is the softmax denominator.
                        ap_ = avp.tile([128, 2, 65], F32, tag="av")
                        for h2 in range(2):
                            for s_ in range(2):
                                nc.tensor.matmul(
                                    ap_[:, h2, :],
                                    pT[:, h2, s_, :],
                                    vp[:, (b - 1 + s_) % 8, 2 * g + h2, :],
                                    start=(s_ == 0), stop=(s_ == 1))
                        rr = rrp.tile([128, 2, 1], F32, tag="rr")
                        nc.vector.reciprocal(rr[:], ap_[:, :, 64:65])
                        nc.vector.tensor_tensor(
                            a_sb[:, g * 128:(g + 1) * 128]
                            .rearrange("p (h e) -> p h e", e=64),
                            ap_[:, :, 0:64],
                            rr[:].broadcast_to([128, 2, 64]), MUL)

                    # transpose a to [dim, query] for the out-projection
                    aT = atp.tile([128, 8, 128], BF16, tag="aT")
                    for j in range(2):
                        aq = obp.tile([128, 512], BF16, tag="aq")
                        for jj in range(4):
                            kc = 4 * j + jj
                            nc.tensor.transpose(
                                aq[:, jj * 128:(jj + 1) * 128],
                                a_sb[:, kc * 128:(kc + 1) * 128],
                                identb[:])
                        nc.vector.tensor_copy(
                            aT[:, 4 * j:4 * j + 4, :],
                            aq[:].rearrange("p (c q) -> p c q", q=128))

                    osb = outp.tile([128, D], F32, tag="o")
                    for nh in range(2):
                        op_ = obp.tile([128, 512], F32, tag="op")
                        for kc in range(8):
                            nc.tensor.matmul(
                                op_[:], aT[:, kc, :],
                                wo[:, kc, nh * 512:(nh + 1) * 512],
                                start=(kc == 0), stop=(kc == 7))
                        nc.scalar.copy(
                            osb[:, nh * 512:(nh + 1) * 512], op_[:])
                    nc.sync.dma_start(out[q0:q0 + 128, :], osb[:])

                for ci, (s, L) in enumerate(CHUNKS):
                    emit_chunk(s, L)
                    lo = 1 if ci == 0 else 4 * ci
                    hi = min(4 * ci + 3, NWIN)
                    for b in range(lo, hi + 1):
                        emit_window(b)

    nc.compile()
    return nc


_NC = {}


def _get_nc(reps=1):
    if reps not in _NC:
        _NC[reps] = _build(reps)
    return _NC[reps]


_r = np.arange(128)


def _host_inputs(x, W_qkv, W_out):
    import ml_dtypes
    bf16 = ml_dtypes.bfloat16

    W = np.ascontiguousarray(W_qkv, np.float32).astype(bf16)

    invf = THETA ** (-(np.arange(0, 64, 2) / 64.0))          # [32]
    rows_f = invf[_r % 32]                                   # [128] freq per row
    # sin tiles are indexed by SOURCE row of the rotate (partner r^32 within
    # each 64-row head); dest sign is folded in: +1 when the source is the
    # first half of its head (it feeds the second half), -1 otherwise.
    rows_s = np.where((_r % 64) < 32, 1.0, -1.0)
    mcol = np.arange(512) % 128
    ang = rows_f[:, None] * mcol[None, :]
    ropes = np.stack([
        SCALE * np.cos(ang),
        SCALE * (rows_s[:, None] * np.sin(ang)),
        np.cos(ang),
        rows_s[:, None] * np.sin(ang),
    ]).astype(bf16)

    a128 = rows_f * 128.0
    r128 = np.stack([
        np.cos(a128)[:, None],
        (rows_s * np.sin(a128))[:, None],
    ]).astype(np.float32)

    # {0,1} band masks in transposed [key j, query i] orientation.
    jj = np.arange(128)[:, None]
    ii = np.arange(128)[None, :]
    m_prev = (ii <= jj).astype(np.float32)    # prev-window keys
    m_cur = (ii >= jj).astype(np.float32)     # current-window keys
    zero = np.zeros_like(m_prev)
    # [variant, h2, slice, j, i]
    mA = np.stack([np.stack([zero, m_cur]), np.stack([zero, m_cur])])
    mB = np.stack([np.stack([m_prev, m_cur]), np.stack([m_prev, m_cur])])
    masks = np.stack([mA, mB]).transpose(0, 3, 1, 2, 4).astype(bf16)
    # masks shape: [2, 128(j), 2(h2), 2(s), 128(i)]

    in_maps = []
    for c in range(NCORES):
        bi, hi = c // 2, c % 2
        xsh = np.empty((NT, D), np.float32)
        if hi == 0:
            xsh[:WS] = 0.0
            xsh[WS:] = x[bi, 0:HALF]
            mvar = masks[np.array([0, 1])]
        else:
            xsh[:] = x[bi, HALF - WS: N]
            mvar = masks[np.array([1, 1])]
        in_maps.append({
            "xs": xsh,
            "wqkv": W,
            "wout": np.ascontiguousarray(W_out, np.float32).astype(bf16),
            "ropes": ropes,
            "r128": r128,
            "masks": mvar,
        })
    return in_maps


def kernel(x, W_qkv, W_out):
    x = np.asarray(x, np.float32)
    nc = _get_nc()
    in_maps = _host_inputs(x, W_qkv, W_out)
    res = run_bass_kernel_spmd(nc, in_maps, list(range(NCORES)))
    outf = np.empty((B, N, D), np.float32)
    for c in range(NCORES):
        bi, hi = c // 2, c % 2
        outf[bi, hi * HALF:(hi + 1) * HALF] = res.results[c]["out"]
    return outf
